# revision 39
# baseline (speedup 1.0000x reference)
"""MLA (multi-head latent attention) forward on 8 TRN2 NeuronCores.

Sharding: 2-way data-parallel over batch x 4-way tensor-parallel over heads.
Core c handles batch b=c//4 and heads 4g..4g+3 where g=c%4. The host sums the
4 partial outputs per batch (the o_proj contribution of each head group).

Q path: RMSNorm is a per-token scalar, so it commutes with the B-projection:
q = rstd(x@q_a) * (x @ (q_a@q_b)). The folded W_q = q_a@q_b is precomputed on
the host and sharded by head (768 cols/core), killing the replicated q_a
matmul. rstd needs ||x@q_a||^2 over the full 1536-dim lora space; each core
computes a 384-col shard of it -- in fp8 with DoubleRow matmuls (2 k-tiles
per instruction, 2x bf16 rate): the norm averages 1536 squared terms, so fp8
quantization noise cancels to ~1e-3. Everything feeding scores/values stays
bf16 (softmax amplifies absolute score error by e^|s|, |s|~6, so fp8 there
costs 3e-2..3e-1 of rel err -- measured, not viable).

KV path: the 512-dim latent projection is column-sharded 4 ways (128 cols per
core) and AllGathered (512KB/rank) across the batch group; the rope slice is
per-head so it stays local. Both norm sum-of-squares partial vectors ride one
16KB AllReduce. All collectives overlap the folded-Wq stage.

Layout: activations are feature-major ([feature, token]) so every matmul
contracts over the partition dim; x arrives pre-transposed/pre-tiled from the
host. Scores are computed transposed (s[tk, tq]) so softmax needs no
max-subtraction (scores are bounded ~6) and P@V contracts naturally. The
causal diagonal blocks are processed triangularly (only query cols >= key
tile). Softmax denominators are accumulated on VectorE from the exp tiles and
partition-reduced on GpSimd (PartitionAllReduce), keeping the PE stream free
of the old fp32 ones-matmuls (fp32 matmul = 4 cyc/row). rstd broadcasts
likewise use GpSimd PartitionBroadcast instead of 1-row fp32 matmuls. Rope q
is stored zero-padded per head in full 128 partitions: mixed 128/64-partition
accumulation pairs force a PE config switch per matmul (~+120ns measured), so
the rope QK matmul contracts the full pair-packed 128 partitions against a
zero-masked rhs. RMSNorm scaling is applied at copy-out of the projected
tensors. Weights are pre-tiled on the host so every weight DMA is contiguous.
"""

import numpy as np
import ml_dtypes

B, T, HIDDEN = 2, 2048, 2048
NUM_HEADS = 16
QK_NOPE, QK_ROPE, HEAD_DIM, V_HEAD = 128, 64, 192, 128
KV_LORA, Q_LORA = 512, 1536
EPS = 1e-6
NCORES = 8
HPC = 4  # heads per core

KC = HIDDEN // 128
TT = T // 128
TQ = T // 512
NS = Q_LORA // 4 // 128  # norm-shard col-tiles (3)
NL = KV_LORA // 128
NR = HPC * QK_ROPE // 128  # rope col-tiles (2)

BF16 = ml_dtypes.bfloat16
FP8 = ml_dtypes.float8_e4m3fn

_CACHE = {}


def _build():
    import concourse.bass as bass
    import concourse.tile as tile
    from concourse import bacc, mybir, bass_isa
    from concourse.bass import ts

    f32 = mybir.dt.float32
    bf = mybir.dt.bfloat16
    f8 = mybir.dt.float8e4
    AF = mybir.ActivationFunctionType
    DR = mybir.MatmulPerfMode.DoubleRow
    RADD = bass_isa.ReduceOp.add
    GROUPS = [[0, 1, 2, 3], [4, 5, 6, 7]]

    nc = bacc.Bacc(
        "TRN2",
        target_bir_lowering=False,
        debug=False,
        enable_asserts=True,
        num_devices=NCORES,
    )

    def din(name, shape, dt=bf):
        return nc.dram_tensor(name, shape, dt, kind="ExternalInput").ap()

    # weights pre-tiled on host: contiguous per-tile DMA loads
    x_ap = din("x", [TQ, 128, KC * 512])              # host-pretransposed rows
    qnw_ap = din("qnw8", [NS, 128, KC, 128], f8)      # fp8 norm-shard of q_a
    wq_ap = din("wq", [2, 128, KC, 2 * HEAD_DIM])     # folded q_a@q_b, pair:
    kvaw_ap = din("kvaw", [1 + NR, 128, KC, 128])     # [lat shard | ropes]
    kvbw_ap = din("kvbw", [128, NL, HPC * (QK_NOPE + V_HEAD)])
    ow_ap = din("ow", [128, HPC, HIDDEN])
    mask_ap = din("mask", [128, 128])                 # 0/1 causal block (bf16)
    out_ap = nc.dram_tensor("out", [HIDDEN, T], bf, kind="ExternalOutput").ap()

    def eng(idx):
        return nc.scalar if idx % 2 else nc.vector

    def copy(e, out, in_):
        if e is nc.scalar:
            nc.scalar.copy(out, in_)
        else:
            nc.vector.tensor_copy(out, in_)

    with tile.TileContext(nc) as tc:
        with tc.tile_pool(name="consts", bufs=1) as consts, \
             tc.tile_pool(name="dram", bufs=1, space="DRAM") as dram, \
             tc.tile_pool(name="act", bufs=1) as act, \
             tc.tile_pool(name="owp", bufs=1) as powp:

            mask = consts.tile([128, 128], bf)
            nc.sync.dma_start(out=mask, in_=mask_ap)
            ones128 = consts.tile([128, 128], bf)
            nc.vector.memset(ones128, 1.0)
            eps1 = consts.tile([128, 1], f32)
            nc.vector.memset(eps1, EPS)

            ropes = act.tile([128, NR, T], bf)   # raw rope keys (2 heads/tile)
            rq_b = act.tile([128, T], f32)
            rkv_b = act.tile([128, T], f32)
            rkvT = act.tile([128, TT], f32)
            qn = act.tile([128, HPC, T], bf)
            qrp = act.tile([128, HPC, T], bf)    # per-head rope q, zero-padded
            nc.vector.memset(qrp, 0.0)

            # AllReduce rows: 0 = q-norm partial sumsq, 1 = kv-latent partial
            # sumsq (each core's 128-col shard) — the reduced row 1 IS the
            # full 512-dim latent sumsq, so no post-AllGather recompute.
            # Quarter-major: each quarter rides its own tiny AllReduce,
            # dispatched as soon as that quarter's partials exist, so the
            # rstd chains never wait on the big AllGather.
            ar_in = dram.tile([TQ, 2, 512], f32)
            ar_out = dram.tile([TQ, 2, 512], f32)
            ag_in = dram.tile([128, T], bf)
            ag_out = dram.tile([NL, 128, T], bf)
            rkv_d = dram.tile([1, T], f32)

            # ---- Stages N/KV/QF need xTq alive; freed before attention
            with tc.tile_pool(name="stageA", bufs=1) as pA, \
                 tc.tile_pool(name="wa", bufs=4) as pwa, \
                 tc.tile_pool(name="wD", bufs=1) as pw, \
                 tc.tile_pool(name="pB", bufs=1) as pB, \
                 tc.tile_pool(name="px8", bufs=1) as px8, \
                 tc.tile_pool(name="psumA", bufs=1, space="PSUM") as psumA:
                # x spreads over all 3 DMA queues every quarter; the weight
                # loads are wedged between quarters by need-time. The front
                # is DMA-bandwidth-bound (~170GB/s aggregate over 16 shared
                # engines), so emission order ~= priority.
                def load_xq(quarter):
                    xt = pA.tile([128, KC, 512], bf, name=f"xT{quarter}")
                    if quarter == 0:
                        qs = (nc.sync, nc.scalar, nc.sync, nc.scalar)
                    else:
                        qs = (nc.sync, nc.scalar, nc.gpsimd, nc.gpsimd)
                    for g, q in enumerate(qs):
                        q.dma_start(
                            out=xt[:, 4 * g:4 * g + 4, :],
                            in_=x_ap[quarter, :, ts(g, 4 * 512)],
                        )
                    return xt

                # latent-shard weight + qnw8 lead the gpsimd queue (needed by
                # lat(0)/N(0)); x quarter 0 on sync+scalar; quarters 1-3 put
                # g2/g3 on gpsimd so no single queue carries more than ~1/3
                # of the front traffic
                kva_tiles = [None, None, None]

                def load_kva(n, q):
                    wa = pwa.tile([128, KC, 128], bf, tag="wa", bufs=3)
                    q.dma_start(out=wa, in_=kvaw_ap[n])
                    kva_tiles[n] = wa

                load_kva(0, nc.gpsimd)
                qnw_tiles = []
                for n in range(NS):
                    wa = pwa.tile([128, KC, 128], f8, tag="w8", bufs=3)
                    nc.gpsimd.dma_start(out=wa, in_=qnw_ap[n])
                    qnw_tiles.append(wa)
                xTq = [load_xq(0)]
                load_kva(1, nc.sync)
                load_kva(2, nc.scalar)
                wq_tiles = []
                wq0 = pw.tile([128, KC, 2 * HEAD_DIM], bf, tag="wq",
                              bufs=2, name="wq0")
                nc.sync.dma_start(out=wq0[:, 0:KC // 2, :],
                                  in_=wq_ap[0, :, 0:KC // 2, :])
                nc.scalar.dma_start(out=wq0[:, KC // 2:KC, :],
                                    in_=wq_ap[0, :, KC // 2:KC, :])
                wq_tiles.append(wq0)
                xTq += [load_xq(q) for q in range(1, TQ)]
                wq1 = pw.tile([128, KC, 2 * HEAD_DIM], bf, tag="wq",
                              bufs=2, name="wq1")
                nc.scalar.dma_start(out=wq1, in_=wq_ap[1])
                wq_tiles.append(wq1)
                kvbw = powp.tile([128, NL, HPC * (QK_NOPE + V_HEAD)], bf)
                nc.sync.dma_start(out=kvbw, in_=kvbw_ap)

                # ~40 back-to-back dummy matmuls warm the PE clock (HAM)
                # while the first x/weight DMAs stream in
                warm = psumA.tile([128, 128], f32, tag="warm", bufs=1)
                for w in range(56):
                    nc.tensor.matmul(out=warm, lhsT=ones128, rhs=ones128,
                                     start=(w == 0), stop=(w == 55))

                # ---- Front pipeline, one x-quarter at a time so compute
                # tracks the DMA arrival: latent shard (its sumsq rides the
                # AllReduce as row 1), stage N (fp8 DoubleRow, 2 k-tiles per
                # instruction, x cast to fp8 on Vector/Scalar per 4-ktile
                # group as each DMA lands), rope slices, then QF head-pair 0.
                # The AllGather dispatches right after the last latent write,
                # the AllReduce right after the last stage-N write, so both
                # overlap QF; QF pair 1 runs as a second sweep.
                def qf_quarter(pair, t):
                    wq = wq_tiles[pair]
                    for sub in range(3):  # nope0 | nope1 | rope pair
                        ps = psumA.tile([128, 512], f32, tag="psm", bufs=4)
                        for kk in range(KC):
                            nc.tensor.matmul(
                                out=ps,
                                lhsT=wq[:, kk, ts(sub, 128)],
                                rhs=xTq[t][:, kk, :],
                                start=(kk == 0),
                                stop=(kk == KC - 1),
                            )
                        if sub < 2:
                            nc.scalar.copy(
                                qn[:, 2 * pair + sub, ts(t, 512)], ps)
                        else:
                            # rope halves land zero-padded per head
                            nc.scalar.copy(
                                qrp[0:64, 2 * pair, ts(t, 512)],
                                ps[0:64, :])
                            nc.scalar.copy(
                                qrp[64:128, 2 * pair + 1, ts(t, 512)],
                                ps[64:128, :])

                for t in range(TQ):
                    x8t = px8.tile([128, KC, 512], f8, tag="x8", bufs=2)
                    for g in range(4):
                        ceng = nc.vector if g % 2 == 0 else nc.scalar
                        copy(ceng, x8t[:, 4 * g:4 * g + 4, :],
                             xTq[t][:, 4 * g:4 * g + 4, :])

                    # latent shard quarter (+ its sumsq partial -> ar row 1)
                    psm = psumA.tile([128, 512], f32, tag="psm", bufs=4)
                    for kk in range(KC):
                        nc.tensor.matmul(
                            out=psm,
                            lhsT=kva_tiles[0][:, kk, :],
                            rhs=xTq[t][:, kk, :],
                            start=(kk == 0),
                            stop=(kk == KC - 1),
                        )
                    lst = pB.tile([128, 512], bf, tag="lst", bufs=2)
                    nc.scalar.copy(lst, psm)
                    nc.gpsimd.dma_start(ag_in[:, ts(t, 512)], lst)
                    sqkv = pB.tile([128, 512], bf, tag="sq", bufs=2)
                    nc.scalar.activation(out=sqkv, in_=psm, func=AF.Square)
                    psk = psumA.tile([128, 512], f32, tag="psd", bufs=2)
                    nc.tensor.matmul(out=psk, lhsT=ones128, rhs=sqkv,
                                     start=True, stop=True)
                    sst = pB.tile([1, 512], f32, tag="sst", bufs=2)
                    nc.scalar.copy(sst, psk[0:1, :])
                    nc.gpsimd.dma_start(ar_in[t, 1:2, :], sst)

                    # stage N quarter
                    psd = psumA.tile([128, 512], f32, tag="psd", bufs=2)
                    for n in range(NS):
                        psm = psumA.tile([128, 512], f32, tag="psm", bufs=4)
                        for kk in range(0, KC, 2):
                            nc.tensor.matmul(
                                out=psm,
                                lhsT=qnw_tiles[n][:, kk:kk + 2, :],
                                rhs=x8t[:, kk:kk + 2, :],
                                start=(kk == 0),
                                stop=(kk == KC - 2),
                                perf_mode=DR,
                            )
                        sq = pB.tile([128, 512], bf, tag="sq", bufs=2)
                        nc.scalar.activation(out=sq, in_=psm, func=AF.Square)
                        nc.tensor.matmul(
                            out=psd, lhsT=ones128, rhs=sq,
                            start=(n == 0), stop=(n == NS - 1),
                        )
                    sst = pB.tile([1, 512], f32, tag="sst", bufs=2)
                    nc.scalar.copy(sst, psd[0:1, :])
                    nc.gpsimd.dma_start(ar_in[t, 0:1, :], sst)

                    # this quarter's 4KB AllReduce fires immediately: its
                    # trigger depends only on this quarter's partials, so
                    # every rstd chain has data long before QF ends. The big
                    # AllGather dispatches only after the LAST AllReduce --
                    # a collective queued behind the AllGather on the CC core
                    # eats its rank skew (~20us observed) on top of ~30us of
                    # data movement.
                    nc.gpsimd.collective_compute(
                        "AllReduce",
                        mybir.AluOpType.add,
                        replica_groups=GROUPS,
                        ins=[ar_in[t].opt()],
                        outs=[ar_out[t].opt()],
                    )
                    if t == TQ - 1:
                        nc.gpsimd.collective_compute(
                            "AllGather",
                            mybir.AluOpType.bypass,
                            replica_groups=GROUPS,
                            ins=[ag_in.opt()],
                            outs=[ag_out.opt()],
                        )

                    # rope slices quarter
                    for n in range(1, 1 + NR):
                        psm = psumA.tile([128, 512], f32, tag="psm", bufs=4)
                        for kk in range(KC):
                            nc.tensor.matmul(
                                out=psm,
                                lhsT=kva_tiles[n][:, kk, :],
                                rhs=xTq[t][:, kk, :],
                                start=(kk == 0),
                                stop=(kk == KC - 1),
                            )
                        nc.scalar.copy(ropes[:, n - 1, ts(t, 512)], psm)

                    # QF head-pair 0 for this quarter
                    qf_quarter(0, t)

                # ---- QF head-pair 1 sweep, with the rstd section wedged in
                # after quarter 1 so its DMA->sqrt->recip->broadcast chains
                # (GpSimd/Scalar/Vector, ~25us of serialized latency) fully
                # overlap the remaining QF matmuls on the PE
                qf_quarter(1, 0)
                qf_quarter(1, 1)

                # rstds from the AllReduced sumsq rows: sqrt + recip on the
                # 1-row sums, then GpSimd PartitionBroadcast; rkv also lands
                # in DRAM to come back transposed (rkvT) for v-scaling.
                # t-major so quarters whose AllReduce finished first flow
                # without waiting on the later ones.
                for t in range(TQ):
                    for row, rb in ((0, rq_b), (1, rkv_b)):
                        lora = Q_LORA if row == 0 else KV_LORA
                        rs1 = pB.tile([1, 512], f32, tag="sst", bufs=2)
                        nc.scalar.dma_start(rs1[:], ar_out[t, row:row + 1, :])
                        t1 = pB.tile([1, 512], f32, tag="t1", bufs=1)
                        nc.scalar.activation(
                            out=t1, in_=rs1, func=AF.Sqrt, bias=eps1[0:1, :],
                            scale=1.0 / lora,
                        )
                        r1 = pB.tile([1, 512], f32, tag="r1", bufs=2)
                        nc.vector.reciprocal_approx_fast(out=r1, in_=t1)
                        nc.gpsimd.partition_broadcast(
                            rb[:, ts(t, 512)], r1, channels=128)
                        if row == 1:
                            nc.gpsimd.dma_start(rkv_d[0:1, ts(t, 512)], r1)
                nc.gpsimd.dma_start(
                    out=rkvT,
                    in_=rkv_d.rearrange("o (tt p) -> (o p) tt", p=128),
                )

                qf_quarter(1, 2)
                qf_quarter(1, 3)

                for t in range(TQ):
                    for h in range(HPC):
                        nc.vector.tensor_mul(
                            qn[:, h, ts(t, 512)], qn[:, h, ts(t, 512)],
                            rq_b[:, ts(t, 512)],
                        )
                    for h in range(HPC):
                        hh = 64 * (h % 2)
                        nc.vector.tensor_mul(
                            qrp[hh:hh + 64, h, ts(t, 512)],
                            qrp[hh:hh + 64, h, ts(t, 512)],
                            rq_b[hh:hh + 64, ts(t, 512)],
                        )

            # ---- Stages D/E/F
            with tc.tile_pool(name="att", bufs=1) as patt, \
                 tc.tile_pool(name="trans", bufs=3) as trans, \
                 tc.tile_pool(name="owd", bufs=1) as powd, \
                 tc.tile_pool(name="psumD", bufs=1, space="PSUM") as psumD:
                kn = patt.tile([128, HPC, T], bf)
                vv = patt.tile([128, TT, HPC * V_HEAD], bf)
                xlat = patt.tile([128, NL, T], bf)   # AllGathered raw latent
                for kk in range(NL):
                    # halves on sync+scalar; gpsimd stays free for the
                    # rstd broadcasts and then the o_w load
                    nc.sync.dma_start(out=xlat[:, kk, 0:1024],
                                      in_=ag_out[kk][:, 0:1024])
                    nc.scalar.dma_start(out=xlat[:, kk, 1024:2048],
                                        in_=ag_out[kk][:, 1024:2048])
                ow = powd.tile([128, HPC, HIDDEN], bf)
                nc.gpsimd.dma_start(out=ow, in_=ow_ap)

                # ---- Stage D: k_nope (x rstd_kv), v (x rstd_kv)
                for h in range(HPC):
                    for t in range(TQ):
                        ps3 = psumD.tile([128, 512], f32, tag="psm", bufs=2)
                        for kk in range(NL):
                            nc.tensor.matmul(
                                out=ps3,
                                lhsT=kvbw[:, kk, ts(h, 256)][:, 0:128],
                                rhs=xlat[:, kk, ts(t, 512)],
                                start=(kk == 0),
                                stop=(kk == NL - 1),
                            )
                        nc.vector.tensor_mul(
                            kn[:, h, ts(t, 512)], ps3, rkv_b[:, ts(t, 512)]
                        )
                vcols = kvbw.rearrange(
                    "p kk (h two dv) -> p kk h two dv", h=HPC, two=2
                )
                for tt in range(TT):
                    psv = psumD.tile([128, 512], f32, tag="psm", bufs=2)
                    for kk in range(NL):
                        nc.tensor.matmul(
                            out=psv,
                            lhsT=xlat[:, kk, ts(tt, 128)],
                            rhs=vcols[:, kk, :, 1, :],
                            start=(kk == 0),
                            stop=(kk == NL - 1),
                        )
                    nc.scalar.mul(vv[:, tt, :], psv, mul=rkvT[:, tt:tt + 1])

                # ---- Stage E+F: causal attention; o_proj one chunk behind
                with tc.tile_pool(name="attn_i", bufs=2) as pai, \
                     tc.tile_pool(name="ob", bufs=2) as pob, \
                     tc.tile_pool(name="rdb", bufs=2) as prdb:
                    attn_tiles = {}

                    def attention_chunk(i):
                        attn_i = pai.tile([128, HPC, 512], bf, tag="attn_i", bufs=2)
                        for h in range(HPC):
                            nj = 4 * i + 4
                            pso = psumD.tile([128, 512], f32, tag="pso", bufs=3)
                            # exp-sum accumulated on VectorE in bf16 (2x DVE
                            # rate; the partition-reduce upcasts to f32);
                            # GpSimd turns it into the softmax denominator
                            acc = prdb.tile([128, 512], bf, tag="acc", bufs=2)

                            def consume_batch(batch, last):
                                for jc, exc, qb in batch:
                                    nc.tensor.matmul(
                                        out=pso[:, qb:512],
                                        lhsT=vv[:, jc, ts(h, V_HEAD)],
                                        rhs=exc[:, qb:512],
                                        start=(jc == 0),
                                        stop=(last and jc == batch[-1][0]),
                                    )

                            pending = []
                            for j in range(nj):
                                off = j * 128 - i * 512
                                qb = max(0, off)  # 1st query col this key sees
                                pss = psumD.tile([128, 512], f32, tag="pss", bufs=3)
                                nc.tensor.matmul(
                                    out=pss[:, qb:512],
                                    lhsT=kn[:, h, ts(j, 128)],
                                    rhs=qn[:, h, i * 512 + qb:(i + 1) * 512],
                                    start=True,
                                    stop=False,
                                )
                                nc.tensor.matmul(
                                    out=pss[:, qb:512],
                                    lhsT=ropes[:, h // 2, ts(j, 128)],
                                    rhs=qrp[:, h, i * 512 + qb:(i + 1) * 512],
                                    start=False,
                                    stop=True,
                                )
                                if len(pending) == 4:
                                    consume_batch(pending, False)
                                    pending = []
                                ex = trans.tile([128, 512], bf, tag="ex", bufs=6)
                                nc.scalar.activation(
                                    out=ex[:, qb:512], in_=pss[:, qb:512],
                                    func=AF.Exp,
                                )
                                if off >= 0:
                                    nc.vector.tensor_mul(
                                        ex[:, qb:qb + 128], ex[:, qb:qb + 128],
                                        mask,
                                    )
                                if j == 0:
                                    nc.vector.tensor_copy(acc, ex)
                                else:
                                    nc.vector.tensor_add(
                                        acc[:, qb:512], acc[:, qb:512],
                                        ex[:, qb:512],
                                    )
                                pending.append((j, ex, qb))
                            if pending:
                                consume_batch(pending, True)

                            # denominator: one cheap bf16 ones-matmul (263ns,
                            # ~1.8us total latency after the last j) -- a
                            # GpSimd PartitionAllReduce here costs 3.8us of
                            # latency and stalls the pso ring every head
                            psdn = psumD.tile([128, 512], f32, tag="psm",
                                              bufs=2)
                            nc.tensor.matmul(out=psdn, lhsT=ones128, rhs=acc,
                                             start=True, stop=True)
                            rdb = prdb.tile([128, 512], f32, tag="rdb", bufs=2)
                            nc.vector.reciprocal_approx_fast(out=rdb, in_=psdn)
                            nc.vector.tensor_mul(attn_i[:, h, :], pso, rdb)
                        attn_tiles[i] = attn_i

                    def oproj_chunk(i):
                        attn_i = attn_tiles[i]
                        for m in range(TT):
                            psf = psumD.tile([128, 512], f32, tag="psm", bufs=2)
                            for kk in range(HPC):
                                nc.tensor.matmul(
                                    out=psf,
                                    lhsT=ow[:, kk, ts(m, 128)],
                                    rhs=attn_i[:, kk, :],
                                    start=(kk == 0),
                                    stop=(kk == HPC - 1),
                                )
                            ob = pob.tile([128, 512], bf, tag="ob", bufs=3)
                            # alternate Scalar/Vector: exp leaves ~half the
                            # ACT engine free during the o_proj windows
                            copy(eng(m), ob, psf)
                            (nc.sync if m % 2 else nc.scalar).dma_start(
                                out=out_ap[ts(m, 128), ts(i, 512)], in_=ob
                            )

                    # big chunks first: the tail then drains on the
                    # smallest attention chunk + its o_proj
                    order = list(range(TQ))[::-1]
                    attention_chunk(order[0])
                    for k in range(1, TQ):
                        attention_chunk(order[k])
                        oproj_chunk(order[k - 1])
                    oproj_chunk(order[-1])

    nc.compile()
    return nc


def _tile_w(w, dt=BF16):
    """[K, N] -> [N/128, 128, K/128, 128] so each col-block loads contiguously."""
    K, N = w.shape
    return np.ascontiguousarray(
        w.astype(dt).reshape(K // 128, 128, N // 128, 128).transpose(2, 1, 0, 3))


def _prep(inputs):
    x = np.asarray(inputs["hidden_states"], np.float32)
    qaw = np.asarray(inputs["q_a_w"], np.float32)
    qalw = np.asarray(inputs["q_a_ln_w"], np.float32)
    qbw = np.asarray(inputs["q_b_w"], np.float32)
    kvaw = np.asarray(inputs["kv_a_w"], np.float32)
    kvlw = np.asarray(inputs["kv_a_ln_w"], np.float32)
    kvbw = np.asarray(inputs["kv_b_w"], np.float32)
    ow = np.asarray(inputs["o_w"], np.float32)

    scale = 1.0 / np.sqrt(np.float32(HEAD_DIM))
    qbw_f = qbw * qalw[:, None] * scale
    wq_full = qaw @ qbw_f  # [HIDDEN, NUM_HEADS*HEAD_DIM] fp32 fold
    kvbw_f = (kvbw * kvlw[:, None]).astype(BF16)

    r = np.arange(128)[:, None]
    j = np.arange(128)[None, :]
    mask = np.where(j >= r, 1.0, 0.0).astype(BF16)

    def lat_tiled(w):  # [KV_LORA, N] -> [128, NL, N] (p, kk, n)
        return np.ascontiguousarray(w.reshape(NL, 128, -1).transpose(1, 0, 2))

    in_maps = []
    for c in range(NCORES):
        b, g = c // 4, c % 4
        # folded W_q for this head group, pair layout [nope0|nope1|ropes]
        wq_g = wq_full[:, g * HPC * HEAD_DIM:(g + 1) * HPC * HEAD_DIM]
        pairs = []
        for pair in range(HPC // 2):
            h0, h1 = 2 * pair, 2 * pair + 1
            cols = np.concatenate([
                wq_g[:, h0 * HEAD_DIM:h0 * HEAD_DIM + QK_NOPE],
                wq_g[:, h1 * HEAD_DIM:h1 * HEAD_DIM + QK_NOPE],
                wq_g[:, h0 * HEAD_DIM + QK_NOPE:(h0 + 1) * HEAD_DIM],
                wq_g[:, h1 * HEAD_DIM + QK_NOPE:(h1 + 1) * HEAD_DIM],
            ], axis=1).astype(BF16)  # [HIDDEN, 384]
            pairs.append(cols.reshape(KC, 128, 384).transpose(1, 0, 2))
        wq_c = np.ascontiguousarray(np.stack(pairs))   # [2, 128, KC, 384]

        # fp8 norm-shard of q_a: 384 cols per core
        qnw8 = _tile_w(np.ascontiguousarray(
            qaw[:, g * 384:(g + 1) * 384]), FP8)

        # kv_a: this core's 128-col latent shard + its 4 heads' rope cols
        kvaw_g = np.concatenate(
            [kvaw[:, g * 128:(g + 1) * 128],
             kvaw[:, KV_LORA + g * HPC * QK_ROPE:
                  KV_LORA + (g + 1) * HPC * QK_ROPE]], axis=1).astype(BF16)

        xt = np.ascontiguousarray(
            x[b].T.astype(BF16).reshape(KC, 128, TQ, 512).transpose(2, 1, 0, 3)
            .reshape(TQ, 128, KC * 512))
        in_maps.append({
            "x": xt,
            "qnw8": qnw8,
            "wq": wq_c,
            "kvaw": _tile_w(kvaw_g),
            "kvbw": lat_tiled(kvbw_f[:, g * HPC * 256:(g + 1) * HPC * 256]),
            "ow": np.ascontiguousarray(
                ow[g * HPC * V_HEAD:(g + 1) * HPC * V_HEAD]
                .astype(BF16).reshape(HPC, 128, HIDDEN).transpose(1, 0, 2)),
            "mask": mask,
        })
    return in_maps


def _ensure_trace_shim():
    """This image lacks antenv.axon_hooks; synthesize it so a trace=True (or
    BASS_TRACE=1) invocation degrades gracefully instead of crashing."""
    import sys
    import types
    try:
        import antenv.axon_hooks  # noqa: F401
        return
    except Exception:
        pass
    try:
        import antenv
        import trn_agent_boot.trn_boot as tb
        hook = tb._ntff_profile_via_ctypes("/opt/axon/libaxon_pjrt.so")
        mod = types.ModuleType("antenv.axon_hooks")
        mod.get_axon_ntff_profile_hook = lambda: hook
        mod.set_axon_ntff_profile_hook = lambda h: None
        antenv.axon_hooks = mod
        sys.modules["antenv.axon_hooks"] = mod
        import concourse.bass_utils as bu
        bu.upload_artifacts = lambda tmpdir: tmpdir
    except Exception:
        pass


def kernel(**inputs):
    from concourse.bass_utils import run_bass_kernel_spmd

    _ensure_trace_shim()
    if "nc" not in _CACHE:
        _CACHE["nc"] = _build()
    nc = _CACHE["nc"]
    in_maps = _prep(inputs)
    try:
        res = run_bass_kernel_spmd(nc, in_maps, core_ids=list(range(NCORES)),
                                   **_CACHE.get("run_kwargs", {}))
    except Exception:
        # transient accelerator faults (e.g. NRT_EXEC_UNIT_UNRECOVERABLE) have
        # been observed after interrupted runs; one retry clears them
        import time
        time.sleep(2)
        res = run_bass_kernel_spmd(nc, in_maps, core_ids=list(range(NCORES)),
                                   **_CACHE.get("run_kwargs", {}))
    _CACHE["last_results"] = res
    out = np.zeros((B, T, HIDDEN), np.float32)
    for c in range(NCORES):
        out[c // 4] += np.asarray(res.results[c]["out"], np.float32).T
    return out


# revision 41
# speedup vs baseline: 1.0226x; 1.0226x over previous
"""MLA (multi-head latent attention) forward on 8 TRN2 NeuronCores.

Sharding: 2-way data-parallel over batch x 4-way tensor-parallel over heads.
Core c handles batch b=c//4 and heads 4g..4g+3 where g=c%4. The host sums the
4 partial outputs per batch (the o_proj contribution of each head group).

Q path: RMSNorm is a per-token scalar, so it commutes with the B-projection:
q = rstd(x@q_a) * (x @ (q_a@q_b)). The folded W_q = q_a@q_b is precomputed on
the host and sharded by head (768 cols/core), killing the replicated q_a
matmul. rstd needs ||x@q_a||^2 over the full 1536-dim lora space; each core
computes a 384-col shard of it -- in fp8 with DoubleRow matmuls (2 k-tiles
per instruction, 2x bf16 rate): the norm averages 1536 squared terms, so fp8
quantization noise cancels to ~1e-3. Everything feeding scores/values stays
bf16 (softmax amplifies absolute score error by e^|s|, |s|~6, so fp8 there
costs 3e-2..3e-1 of rel err -- measured, not viable).

KV path: the 512-dim latent projection is column-sharded 4 ways (128 cols per
core) and AllGathered (512KB/rank) across the batch group; the rope slice is
per-head so it stays local. Both norm sum-of-squares partial vectors ride one
16KB AllReduce. All collectives overlap the folded-Wq stage.

Layout: activations are feature-major ([feature, token]) so every matmul
contracts over the partition dim; x arrives pre-transposed/pre-tiled from the
host. Scores are computed transposed (s[tk, tq]) so softmax needs no
max-subtraction (scores are bounded ~6) and P@V contracts naturally. The
causal diagonal blocks are processed triangularly (only query cols >= key
tile). Softmax denominators are accumulated on VectorE from the exp tiles and
partition-reduced on GpSimd (PartitionAllReduce), keeping the PE stream free
of the old fp32 ones-matmuls (fp32 matmul = 4 cyc/row). rstd broadcasts
likewise use GpSimd PartitionBroadcast instead of 1-row fp32 matmuls. Rope q
is stored zero-padded per head in full 128 partitions: mixed 128/64-partition
accumulation pairs force a PE config switch per matmul (~+120ns measured), so
the rope QK matmul contracts the full pair-packed 128 partitions against a
zero-masked rhs. RMSNorm scaling is applied at copy-out of the projected
tensors. Weights are pre-tiled on the host so every weight DMA is contiguous.
"""

import numpy as np
import ml_dtypes

B, T, HIDDEN = 2, 2048, 2048
NUM_HEADS = 16
QK_NOPE, QK_ROPE, HEAD_DIM, V_HEAD = 128, 64, 192, 128
KV_LORA, Q_LORA = 512, 1536
EPS = 1e-6
NCORES = 8
HPC = 4  # heads per core

KC = HIDDEN // 128
TT = T // 128
TQ = T // 512
NS = Q_LORA // 4 // 128  # norm-shard col-tiles (3)
NL = KV_LORA // 128
NR = HPC * QK_ROPE // 128  # rope col-tiles (2)

BF16 = ml_dtypes.bfloat16
FP8 = ml_dtypes.float8_e4m3fn

_CACHE = {}


def _build():
    import concourse.bass as bass
    import concourse.tile as tile
    from concourse import bacc, mybir, bass_isa
    from concourse.bass import ts

    f32 = mybir.dt.float32
    bf = mybir.dt.bfloat16
    f8 = mybir.dt.float8e4
    AF = mybir.ActivationFunctionType
    DR = mybir.MatmulPerfMode.DoubleRow
    RADD = bass_isa.ReduceOp.add
    GROUPS = [[0, 1, 2, 3], [4, 5, 6, 7]]

    nc = bacc.Bacc(
        "TRN2",
        target_bir_lowering=False,
        debug=False,
        enable_asserts=True,
        num_devices=NCORES,
    )

    def din(name, shape, dt=bf):
        return nc.dram_tensor(name, shape, dt, kind="ExternalInput").ap()

    # weights pre-tiled on host: contiguous per-tile DMA loads
    x_ap = din("x", [TQ, 128, KC * 512])              # host-pretransposed rows
    qnw_ap = din("qnw8", [NS, 128, KC, 128], f8)      # fp8 norm-shard of q_a
    wq_ap = din("wq", [2, 128, KC, 2 * HEAD_DIM])     # folded q_a@q_b, pair:
    kvaw_ap = din("kvaw", [1 + NR, 128, KC, 128])     # [lat shard | ropes]
    kvbw_ap = din("kvbw", [128, NL, HPC * (QK_NOPE + V_HEAD)])
    ow_ap = din("ow", [128, HPC, HIDDEN])
    mask_ap = din("mask", [128, 128])                 # 0/1 causal block (bf16)
    out_ap = nc.dram_tensor("out", [HIDDEN, T], bf, kind="ExternalOutput").ap()

    def eng(idx):
        return nc.scalar if idx % 2 else nc.vector

    def copy(e, out, in_):
        if e is nc.scalar:
            nc.scalar.copy(out, in_)
        else:
            nc.vector.tensor_copy(out, in_)

    with tile.TileContext(nc) as tc:
        with tc.tile_pool(name="consts", bufs=1) as consts, \
             tc.tile_pool(name="dram", bufs=1, space="DRAM") as dram, \
             tc.tile_pool(name="act", bufs=1) as act, \
             tc.tile_pool(name="owp", bufs=1) as powp:

            mask = consts.tile([128, 128], bf)
            nc.sync.dma_start(out=mask, in_=mask_ap)
            ones128 = consts.tile([128, 128], bf)
            nc.vector.memset(ones128, 1.0)
            ones1f = consts.tile([1, 128], f32)
            nc.vector.memset(ones1f, 1.0)
            eps1 = consts.tile([128, 1], f32)
            nc.vector.memset(eps1, EPS)

            ropes = act.tile([128, NR, T], bf)   # raw rope keys (2 heads/tile)
            rq_b = act.tile([128, T], f32)
            rkv_b = act.tile([128, T], f32)
            rkvT = act.tile([128, TT], f32)
            qn = act.tile([128, HPC, T], bf)
            qrp = act.tile([128, HPC, T], bf)    # per-head rope q, zero-padded
            nc.vector.memset(qrp, 0.0)

            # AllReduce rows: 0 = q-norm partial sumsq, 1 = kv-latent partial
            # sumsq (each core's 128-col shard) — the reduced row 1 IS the
            # full 512-dim latent sumsq, so no post-AllGather recompute.
            # Quarter-major: each quarter rides its own tiny AllReduce,
            # dispatched as soon as that quarter's partials exist, so the
            # rstd chains never wait on the big AllGather.
            ar_in = dram.tile([TQ, 2, 512], f32)
            ar_out = dram.tile([TQ, 2, 512], f32)
            ag_in = dram.tile([128, T], bf)
            ag_out = dram.tile([NL, 128, T], bf)
            rkv_d = dram.tile([1, T], f32)

            # ---- Stages N/KV/QF need xTq alive; freed before attention
            with tc.tile_pool(name="stageA", bufs=1) as pA, \
                 tc.tile_pool(name="wa", bufs=4) as pwa, \
                 tc.tile_pool(name="wD", bufs=1) as pw, \
                 tc.tile_pool(name="pB", bufs=1) as pB, \
                 tc.tile_pool(name="px8", bufs=1) as px8, \
                 tc.tile_pool(name="psumA", bufs=1, space="PSUM") as psumA:
                # x spreads over all 3 DMA queues every quarter; the weight
                # loads are wedged between quarters by need-time. The front
                # is DMA-bandwidth-bound (~170GB/s aggregate over 16 shared
                # engines), so emission order ~= priority.
                def load_xq(quarter):
                    xt = pA.tile([128, KC, 512], bf, name=f"xT{quarter}")
                    if quarter == 0:
                        qs = (nc.sync, nc.scalar, nc.sync, nc.scalar)
                    else:
                        qs = (nc.sync, nc.scalar, nc.gpsimd, nc.gpsimd)
                    for g, q in enumerate(qs):
                        q.dma_start(
                            out=xt[:, 4 * g:4 * g + 4, :],
                            in_=x_ap[quarter, :, ts(g, 4 * 512)],
                        )
                    return xt

                # latent-shard weight + qnw8 lead the gpsimd queue (needed by
                # lat(0)/N(0)); x quarter 0 on sync+scalar; quarters 1-3 put
                # g2/g3 on gpsimd so no single queue carries more than ~1/3
                # of the front traffic
                kva_tiles = [None, None, None]

                def load_kva(n, q):
                    wa = pwa.tile([128, KC, 128], bf, tag="wa", bufs=3)
                    q.dma_start(out=wa, in_=kvaw_ap[n])
                    kva_tiles[n] = wa

                load_kva(0, nc.gpsimd)
                qnw_tiles = []
                for n in range(NS):
                    wa = pwa.tile([128, KC, 128], f8, tag="w8", bufs=3)
                    nc.gpsimd.dma_start(out=wa, in_=qnw_ap[n])
                    qnw_tiles.append(wa)
                xTq = [load_xq(0)]
                load_kva(1, nc.sync)
                load_kva(2, nc.scalar)
                wq_tiles = []
                wq0 = pw.tile([128, KC, 2 * HEAD_DIM], bf, tag="wq",
                              bufs=2, name="wq0")
                nc.sync.dma_start(out=wq0[:, 0:KC // 2, :],
                                  in_=wq_ap[0, :, 0:KC // 2, :])
                nc.scalar.dma_start(out=wq0[:, KC // 2:KC, :],
                                    in_=wq_ap[0, :, KC // 2:KC, :])
                wq_tiles.append(wq0)
                xTq += [load_xq(q) for q in range(1, TQ)]
                wq1 = pw.tile([128, KC, 2 * HEAD_DIM], bf, tag="wq",
                              bufs=2, name="wq1")
                nc.scalar.dma_start(out=wq1, in_=wq_ap[1])
                wq_tiles.append(wq1)
                kvbw = powp.tile([128, NL, HPC * (QK_NOPE + V_HEAD)], bf)
                nc.sync.dma_start(out=kvbw, in_=kvbw_ap)

                # ~40 back-to-back dummy matmuls warm the PE clock (HAM)
                # while the first x/weight DMAs stream in
                warm = psumA.tile([128, 128], f32, tag="warm", bufs=1)
                for w in range(56):
                    nc.tensor.matmul(out=warm, lhsT=ones128, rhs=ones128,
                                     start=(w == 0), stop=(w == 55))

                # ---- Front pipeline, one x-quarter at a time so compute
                # tracks the DMA arrival: latent shard (its sumsq rides the
                # AllReduce as row 1), stage N (fp8 DoubleRow, 2 k-tiles per
                # instruction, x cast to fp8 on Vector/Scalar per 4-ktile
                # group as each DMA lands), rope slices, then QF head-pair 0.
                # The AllGather dispatches right after the last latent write,
                # the AllReduce right after the last stage-N write, so both
                # overlap QF; QF pair 1 runs as a second sweep.
                def qf_quarter(pair, t):
                    wq = wq_tiles[pair]
                    for sub in range(3):  # nope0 | nope1 | rope pair
                        ps = psumA.tile([128, 512], f32, tag="psm", bufs=4)
                        for kk in range(KC):
                            nc.tensor.matmul(
                                out=ps,
                                lhsT=wq[:, kk, ts(sub, 128)],
                                rhs=xTq[t][:, kk, :],
                                start=(kk == 0),
                                stop=(kk == KC - 1),
                            )
                        if sub < 2:
                            nc.scalar.copy(
                                qn[:, 2 * pair + sub, ts(t, 512)], ps)
                        else:
                            # rope halves land zero-padded per head
                            nc.scalar.copy(
                                qrp[0:64, 2 * pair, ts(t, 512)],
                                ps[0:64, :])
                            nc.scalar.copy(
                                qrp[64:128, 2 * pair + 1, ts(t, 512)],
                                ps[64:128, :])

                for t in range(TQ):
                    x8t = px8.tile([128, KC, 512], f8, tag="x8", bufs=2)
                    for g in range(4):
                        ceng = nc.vector if g % 2 == 0 else nc.scalar
                        copy(ceng, x8t[:, 4 * g:4 * g + 4, :],
                             xTq[t][:, 4 * g:4 * g + 4, :])

                    # latent shard quarter (+ its sumsq partial -> ar row 1)
                    psm = psumA.tile([128, 512], f32, tag="psm", bufs=4)
                    for kk in range(KC):
                        nc.tensor.matmul(
                            out=psm,
                            lhsT=kva_tiles[0][:, kk, :],
                            rhs=xTq[t][:, kk, :],
                            start=(kk == 0),
                            stop=(kk == KC - 1),
                        )
                    lst = pB.tile([128, 512], bf, tag="lst", bufs=2)
                    nc.scalar.copy(lst, psm)
                    nc.gpsimd.dma_start(ag_in[:, ts(t, 512)], lst)
                    sqkv = pB.tile([128, 512], bf, tag="sq", bufs=2)
                    nc.scalar.activation(out=sqkv, in_=psm, func=AF.Square)
                    psk = psumA.tile([128, 512], f32, tag="psd", bufs=2)
                    nc.tensor.matmul(out=psk, lhsT=ones128, rhs=sqkv,
                                     start=True, stop=True)
                    sst = pB.tile([1, 512], f32, tag="sst", bufs=2)
                    nc.scalar.copy(sst, psk[0:1, :])
                    nc.gpsimd.dma_start(ar_in[t, 1:2, :], sst)

                    # stage N quarter
                    psd = psumA.tile([128, 512], f32, tag="psd", bufs=2)
                    for n in range(NS):
                        psm = psumA.tile([128, 512], f32, tag="psm", bufs=4)
                        for kk in range(0, KC, 2):
                            nc.tensor.matmul(
                                out=psm,
                                lhsT=qnw_tiles[n][:, kk:kk + 2, :],
                                rhs=x8t[:, kk:kk + 2, :],
                                start=(kk == 0),
                                stop=(kk == KC - 2),
                                perf_mode=DR,
                            )
                        sq = pB.tile([128, 512], bf, tag="sq", bufs=2)
                        nc.scalar.activation(out=sq, in_=psm, func=AF.Square)
                        nc.tensor.matmul(
                            out=psd, lhsT=ones128, rhs=sq,
                            start=(n == 0), stop=(n == NS - 1),
                        )
                    sst = pB.tile([1, 512], f32, tag="sst", bufs=2)
                    nc.scalar.copy(sst, psd[0:1, :])
                    nc.gpsimd.dma_start(ar_in[t, 0:1, :], sst)

                    # this quarter's 4KB AllReduce fires immediately: its
                    # trigger depends only on this quarter's partials, so
                    # every rstd chain has data long before QF ends. The big
                    # AllGather dispatches only after the LAST AllReduce --
                    # a collective queued behind the AllGather on the CC core
                    # eats its rank skew (~20us observed) on top of ~30us of
                    # data movement.
                    nc.gpsimd.collective_compute(
                        "AllReduce",
                        mybir.AluOpType.add,
                        replica_groups=GROUPS,
                        ins=[ar_in[t].opt()],
                        outs=[ar_out[t].opt()],
                    )
                    if t == TQ - 1:
                        nc.gpsimd.collective_compute(
                            "AllGather",
                            mybir.AluOpType.bypass,
                            replica_groups=GROUPS,
                            ins=[ag_in.opt()],
                            outs=[ag_out.opt()],
                        )

                    # rope slices quarter
                    for n in range(1, 1 + NR):
                        psm = psumA.tile([128, 512], f32, tag="psm", bufs=4)
                        for kk in range(KC):
                            nc.tensor.matmul(
                                out=psm,
                                lhsT=kva_tiles[n][:, kk, :],
                                rhs=xTq[t][:, kk, :],
                                start=(kk == 0),
                                stop=(kk == KC - 1),
                            )
                        nc.scalar.copy(ropes[:, n - 1, ts(t, 512)], psm)

                    # QF head-pair 0 for this quarter
                    qf_quarter(0, t)

                # ---- QF head-pair 1 sweep, with the rstd section wedged in
                # after quarter 1 so its DMA->sqrt->recip->broadcast chains
                # (GpSimd/Scalar/Vector, ~25us of serialized latency) fully
                # overlap the remaining QF matmuls on the PE
                qf_quarter(1, 0)
                qf_quarter(1, 1)

                # rstds from the AllReduced sumsq rows: a 1-row f32 ones-
                # matmul broadcasts the raw sums (850ns on the PE -- a GpSimd
                # PartitionBroadcast serializes 2-4us per call), then sqrt +
                # recip on full-height tiles so the chains pipeline across
                # PE/Scalar/Vector ~1us apart. t-major so quarters whose
                # AllReduce finished first flow without waiting on the rest.
                for t in range(TQ):
                    for row, rb in ((0, rq_b), (1, rkv_b)):
                        lora = Q_LORA if row == 0 else KV_LORA
                        rs1 = pB.tile([1, 512], f32, tag="sst", bufs=2)
                        nc.scalar.dma_start(rs1[:], ar_out[t, row:row + 1, :])
                        psb = psumA.tile([128, 512], f32, tag="psd", bufs=2)
                        nc.tensor.matmul(out=psb, lhsT=ones1f, rhs=rs1,
                                         start=True, stop=True)
                        t1 = pB.tile([128, 512], f32, tag="t1", bufs=2)
                        nc.scalar.activation(
                            out=t1, in_=psb, func=AF.Sqrt, bias=eps1,
                            scale=1.0 / lora,
                        )
                        nc.vector.reciprocal_approx_fast(
                            out=rb[:, ts(t, 512)], in_=t1)
                        if row == 1:
                            nc.gpsimd.dma_start(rkv_d[0:1, ts(t, 512)],
                                                rb[0:1, ts(t, 512)])
                nc.gpsimd.dma_start(
                    out=rkvT,
                    in_=rkv_d.rearrange("o (tt p) -> (o p) tt", p=128),
                )

                qf_quarter(1, 2)
                qf_quarter(1, 3)

                for t in range(TQ):
                    for h in range(HPC):
                        nc.vector.tensor_mul(
                            qn[:, h, ts(t, 512)], qn[:, h, ts(t, 512)],
                            rq_b[:, ts(t, 512)],
                        )
                    for h in range(HPC):
                        hh = 64 * (h % 2)
                        nc.vector.tensor_mul(
                            qrp[hh:hh + 64, h, ts(t, 512)],
                            qrp[hh:hh + 64, h, ts(t, 512)],
                            rq_b[hh:hh + 64, ts(t, 512)],
                        )

            # ---- Stages D/E/F
            with tc.tile_pool(name="att", bufs=1) as patt, \
                 tc.tile_pool(name="trans", bufs=3) as trans, \
                 tc.tile_pool(name="owd", bufs=1) as powd, \
                 tc.tile_pool(name="psumD", bufs=1, space="PSUM") as psumD:
                kn = patt.tile([128, HPC, T], bf)
                vv = patt.tile([128, TT, HPC * V_HEAD], bf)
                xlat = patt.tile([128, NL, T], bf)   # AllGathered raw latent
                for kk in range(NL):
                    # halves on sync+scalar; gpsimd stays free for the
                    # rstd broadcasts and then the o_w load
                    nc.sync.dma_start(out=xlat[:, kk, 0:1024],
                                      in_=ag_out[kk][:, 0:1024])
                    nc.scalar.dma_start(out=xlat[:, kk, 1024:2048],
                                        in_=ag_out[kk][:, 1024:2048])
                ow = powd.tile([128, HPC, HIDDEN], bf)
                nc.gpsimd.dma_start(out=ow, in_=ow_ap)

                # ---- Stage D: k_nope (x rstd_kv), v (x rstd_kv)
                for h in range(HPC):
                    for t in range(TQ):
                        ps3 = psumD.tile([128, 512], f32, tag="psm", bufs=2)
                        for kk in range(NL):
                            nc.tensor.matmul(
                                out=ps3,
                                lhsT=kvbw[:, kk, ts(h, 256)][:, 0:128],
                                rhs=xlat[:, kk, ts(t, 512)],
                                start=(kk == 0),
                                stop=(kk == NL - 1),
                            )
                        nc.vector.tensor_mul(
                            kn[:, h, ts(t, 512)], ps3, rkv_b[:, ts(t, 512)]
                        )
                vcols = kvbw.rearrange(
                    "p kk (h two dv) -> p kk h two dv", h=HPC, two=2
                )
                for tt in range(TT):
                    psv = psumD.tile([128, 512], f32, tag="psm", bufs=2)
                    for kk in range(NL):
                        nc.tensor.matmul(
                            out=psv,
                            lhsT=xlat[:, kk, ts(tt, 128)],
                            rhs=vcols[:, kk, :, 1, :],
                            start=(kk == 0),
                            stop=(kk == NL - 1),
                        )
                    nc.scalar.mul(vv[:, tt, :], psv, mul=rkvT[:, tt:tt + 1])

                # ---- Stage E+F: causal attention; o_proj one chunk behind
                with tc.tile_pool(name="attn_i", bufs=2) as pai, \
                     tc.tile_pool(name="ob", bufs=2) as pob, \
                     tc.tile_pool(name="rdb", bufs=2) as prdb:
                    attn_tiles = {}

                    def attention_chunk(i):
                        attn_i = pai.tile([128, HPC, 512], bf, tag="attn_i", bufs=2)
                        for h in range(HPC):
                            nj = 4 * i + 4
                            pso = psumD.tile([128, 512], f32, tag="pso", bufs=3)
                            # exp-sum accumulated on VectorE in bf16 (2x DVE
                            # rate; the partition-reduce upcasts to f32);
                            # GpSimd turns it into the softmax denominator
                            acc = prdb.tile([128, 512], bf, tag="acc", bufs=2)

                            def consume_batch(batch, last):
                                for jc, exc, qb in batch:
                                    nc.tensor.matmul(
                                        out=pso[:, qb:512],
                                        lhsT=vv[:, jc, ts(h, V_HEAD)],
                                        rhs=exc[:, qb:512],
                                        start=(jc == 0),
                                        stop=(last and jc == batch[-1][0]),
                                    )

                            pending = []
                            for j in range(nj):
                                off = j * 128 - i * 512
                                qb = max(0, off)  # 1st query col this key sees
                                pss = psumD.tile([128, 512], f32, tag="pss", bufs=3)
                                nc.tensor.matmul(
                                    out=pss[:, qb:512],
                                    lhsT=kn[:, h, ts(j, 128)],
                                    rhs=qn[:, h, i * 512 + qb:(i + 1) * 512],
                                    start=True,
                                    stop=False,
                                )
                                nc.tensor.matmul(
                                    out=pss[:, qb:512],
                                    lhsT=ropes[:, h // 2, ts(j, 128)],
                                    rhs=qrp[:, h, i * 512 + qb:(i + 1) * 512],
                                    start=False,
                                    stop=True,
                                )
                                if len(pending) == 4:
                                    consume_batch(pending, False)
                                    pending = []
                                ex = trans.tile([128, 512], bf, tag="ex", bufs=6)
                                nc.scalar.activation(
                                    out=ex[:, qb:512], in_=pss[:, qb:512],
                                    func=AF.Exp,
                                )
                                if off >= 0:
                                    nc.vector.tensor_mul(
                                        ex[:, qb:qb + 128], ex[:, qb:qb + 128],
                                        mask,
                                    )
                                if j == 0:
                                    nc.vector.tensor_copy(acc, ex)
                                else:
                                    nc.vector.tensor_add(
                                        acc[:, qb:512], acc[:, qb:512],
                                        ex[:, qb:512],
                                    )
                                pending.append((j, ex, qb))
                            if pending:
                                consume_batch(pending, True)

                            # denominator: one cheap bf16 ones-matmul (263ns,
                            # ~1.8us total latency after the last j) -- a
                            # GpSimd PartitionAllReduce here costs 3.8us of
                            # latency and stalls the pso ring every head
                            psdn = psumD.tile([128, 512], f32, tag="psm",
                                              bufs=2)
                            nc.tensor.matmul(out=psdn, lhsT=ones128, rhs=acc,
                                             start=True, stop=True)
                            rdb = prdb.tile([128, 512], f32, tag="rdb", bufs=2)
                            nc.vector.reciprocal_approx_fast(out=rdb, in_=psdn)
                            nc.vector.tensor_mul(attn_i[:, h, :], pso, rdb)
                        attn_tiles[i] = attn_i

                    def oproj_chunk(i):
                        attn_i = attn_tiles[i]
                        for m in range(TT):
                            psf = psumD.tile([128, 512], f32, tag="psm", bufs=2)
                            for kk in range(HPC):
                                nc.tensor.matmul(
                                    out=psf,
                                    lhsT=ow[:, kk, ts(m, 128)],
                                    rhs=attn_i[:, kk, :],
                                    start=(kk == 0),
                                    stop=(kk == HPC - 1),
                                )
                            ob = pob.tile([128, 512], bf, tag="ob", bufs=3)
                            # alternate Scalar/Vector: exp leaves ~half the
                            # ACT engine free during the o_proj windows
                            copy(eng(m), ob, psf)
                            (nc.sync if m % 2 else nc.scalar).dma_start(
                                out=out_ap[ts(m, 128), ts(i, 512)], in_=ob
                            )

                    # big chunks first: the tail then drains on the
                    # smallest attention chunk + its o_proj
                    order = list(range(TQ))[::-1]
                    attention_chunk(order[0])
                    for k in range(1, TQ):
                        attention_chunk(order[k])
                        oproj_chunk(order[k - 1])
                    oproj_chunk(order[-1])

    nc.compile()
    return nc


def _tile_w(w, dt=BF16):
    """[K, N] -> [N/128, 128, K/128, 128] so each col-block loads contiguously."""
    K, N = w.shape
    return np.ascontiguousarray(
        w.astype(dt).reshape(K // 128, 128, N // 128, 128).transpose(2, 1, 0, 3))


def _prep(inputs):
    x = np.asarray(inputs["hidden_states"], np.float32)
    qaw = np.asarray(inputs["q_a_w"], np.float32)
    qalw = np.asarray(inputs["q_a_ln_w"], np.float32)
    qbw = np.asarray(inputs["q_b_w"], np.float32)
    kvaw = np.asarray(inputs["kv_a_w"], np.float32)
    kvlw = np.asarray(inputs["kv_a_ln_w"], np.float32)
    kvbw = np.asarray(inputs["kv_b_w"], np.float32)
    ow = np.asarray(inputs["o_w"], np.float32)

    scale = 1.0 / np.sqrt(np.float32(HEAD_DIM))
    qbw_f = qbw * qalw[:, None] * scale
    wq_full = qaw @ qbw_f  # [HIDDEN, NUM_HEADS*HEAD_DIM] fp32 fold
    kvbw_f = (kvbw * kvlw[:, None]).astype(BF16)

    r = np.arange(128)[:, None]
    j = np.arange(128)[None, :]
    mask = np.where(j >= r, 1.0, 0.0).astype(BF16)

    def lat_tiled(w):  # [KV_LORA, N] -> [128, NL, N] (p, kk, n)
        return np.ascontiguousarray(w.reshape(NL, 128, -1).transpose(1, 0, 2))

    in_maps = []
    for c in range(NCORES):
        b, g = c // 4, c % 4
        # folded W_q for this head group, pair layout [nope0|nope1|ropes]
        wq_g = wq_full[:, g * HPC * HEAD_DIM:(g + 1) * HPC * HEAD_DIM]
        pairs = []
        for pair in range(HPC // 2):
            h0, h1 = 2 * pair, 2 * pair + 1
            cols = np.concatenate([
                wq_g[:, h0 * HEAD_DIM:h0 * HEAD_DIM + QK_NOPE],
                wq_g[:, h1 * HEAD_DIM:h1 * HEAD_DIM + QK_NOPE],
                wq_g[:, h0 * HEAD_DIM + QK_NOPE:(h0 + 1) * HEAD_DIM],
                wq_g[:, h1 * HEAD_DIM + QK_NOPE:(h1 + 1) * HEAD_DIM],
            ], axis=1).astype(BF16)  # [HIDDEN, 384]
            pairs.append(cols.reshape(KC, 128, 384).transpose(1, 0, 2))
        wq_c = np.ascontiguousarray(np.stack(pairs))   # [2, 128, KC, 384]

        # fp8 norm-shard of q_a: 384 cols per core
        qnw8 = _tile_w(np.ascontiguousarray(
            qaw[:, g * 384:(g + 1) * 384]), FP8)

        # kv_a: this core's 128-col latent shard + its 4 heads' rope cols
        kvaw_g = np.concatenate(
            [kvaw[:, g * 128:(g + 1) * 128],
             kvaw[:, KV_LORA + g * HPC * QK_ROPE:
                  KV_LORA + (g + 1) * HPC * QK_ROPE]], axis=1).astype(BF16)

        xt = np.ascontiguousarray(
            x[b].T.astype(BF16).reshape(KC, 128, TQ, 512).transpose(2, 1, 0, 3)
            .reshape(TQ, 128, KC * 512))
        in_maps.append({
            "x": xt,
            "qnw8": qnw8,
            "wq": wq_c,
            "kvaw": _tile_w(kvaw_g),
            "kvbw": lat_tiled(kvbw_f[:, g * HPC * 256:(g + 1) * HPC * 256]),
            "ow": np.ascontiguousarray(
                ow[g * HPC * V_HEAD:(g + 1) * HPC * V_HEAD]
                .astype(BF16).reshape(HPC, 128, HIDDEN).transpose(1, 0, 2)),
            "mask": mask,
        })
    return in_maps


def _ensure_trace_shim():
    """This image lacks antenv.axon_hooks; synthesize it so a trace=True (or
    BASS_TRACE=1) invocation degrades gracefully instead of crashing."""
    import sys
    import types
    try:
        import antenv.axon_hooks  # noqa: F401
        return
    except Exception:
        pass
    try:
        import antenv
        import trn_agent_boot.trn_boot as tb
        hook = tb._ntff_profile_via_ctypes("/opt/axon/libaxon_pjrt.so")
        mod = types.ModuleType("antenv.axon_hooks")
        mod.get_axon_ntff_profile_hook = lambda: hook
        mod.set_axon_ntff_profile_hook = lambda h: None
        antenv.axon_hooks = mod
        sys.modules["antenv.axon_hooks"] = mod
        import concourse.bass_utils as bu
        bu.upload_artifacts = lambda tmpdir: tmpdir
    except Exception:
        pass


def kernel(**inputs):
    from concourse.bass_utils import run_bass_kernel_spmd

    _ensure_trace_shim()
    if "nc" not in _CACHE:
        _CACHE["nc"] = _build()
    nc = _CACHE["nc"]
    in_maps = _prep(inputs)
    try:
        res = run_bass_kernel_spmd(nc, in_maps, core_ids=list(range(NCORES)),
                                   **_CACHE.get("run_kwargs", {}))
    except Exception:
        # transient accelerator faults (e.g. NRT_EXEC_UNIT_UNRECOVERABLE) have
        # been observed after interrupted runs; one retry clears them
        import time
        time.sleep(2)
        res = run_bass_kernel_spmd(nc, in_maps, core_ids=list(range(NCORES)),
                                   **_CACHE.get("run_kwargs", {}))
    _CACHE["last_results"] = res
    out = np.zeros((B, T, HIDDEN), np.float32)
    for c in range(NCORES):
        out[c // 4] += np.asarray(res.results[c]["out"], np.float32).T
    return out


# revision 43
# speedup vs baseline: 1.0239x; 1.0013x over previous
"""MLA (multi-head latent attention) forward on 8 TRN2 NeuronCores.

Sharding: 2-way data-parallel over batch x 4-way tensor-parallel over heads.
Core c handles batch b=c//4 and heads 4g..4g+3 where g=c%4. The host sums the
4 partial outputs per batch (the o_proj contribution of each head group).

Q path: RMSNorm is a per-token scalar, so it commutes with the B-projection:
q = rstd(x@q_a) * (x @ (q_a@q_b)). The folded W_q = q_a@q_b is precomputed on
the host and sharded by head (768 cols/core), killing the replicated q_a
matmul. rstd needs ||x@q_a||^2 over the full 1536-dim lora space; each core
computes a 384-col shard of it -- in fp8 with DoubleRow matmuls (2 k-tiles
per instruction, 2x bf16 rate): the norm averages 1536 squared terms, so fp8
quantization noise cancels to ~1e-3. Everything feeding scores/values stays
bf16 (softmax amplifies absolute score error by e^|s|, |s|~6, so fp8 there
costs 3e-2..3e-1 of rel err -- measured, not viable).

KV path: the 512-dim latent projection is column-sharded 4 ways (128 cols per
core) and AllGathered (512KB/rank) across the batch group; the rope slice is
per-head so it stays local. Both norm sum-of-squares partial vectors ride one
16KB AllReduce. All collectives overlap the folded-Wq stage.

Layout: activations are feature-major ([feature, token]) so every matmul
contracts over the partition dim; x arrives pre-transposed/pre-tiled from the
host. Scores are computed transposed (s[tk, tq]) so softmax needs no
max-subtraction (scores are bounded ~6) and P@V contracts naturally. The
causal diagonal blocks are processed triangularly (only query cols >= key
tile). Softmax denominators are accumulated on VectorE from the exp tiles and
partition-reduced on GpSimd (PartitionAllReduce), keeping the PE stream free
of the old fp32 ones-matmuls (fp32 matmul = 4 cyc/row). rstd broadcasts
likewise use GpSimd PartitionBroadcast instead of 1-row fp32 matmuls. Rope q
is stored zero-padded per head in full 128 partitions: mixed 128/64-partition
accumulation pairs force a PE config switch per matmul (~+120ns measured), so
the rope QK matmul contracts the full pair-packed 128 partitions against a
zero-masked rhs. RMSNorm scaling is applied at copy-out of the projected
tensors. Weights are pre-tiled on the host so every weight DMA is contiguous.
"""

import numpy as np
import ml_dtypes

B, T, HIDDEN = 2, 2048, 2048
NUM_HEADS = 16
QK_NOPE, QK_ROPE, HEAD_DIM, V_HEAD = 128, 64, 192, 128
KV_LORA, Q_LORA = 512, 1536
EPS = 1e-6
NCORES = 8
HPC = 4  # heads per core

KC = HIDDEN // 128
TT = T // 128
TQ = T // 512
NS = Q_LORA // 4 // 128  # norm-shard col-tiles (3)
NL = KV_LORA // 128
NR = HPC * QK_ROPE // 128  # rope col-tiles (2)

BF16 = ml_dtypes.bfloat16
FP8 = ml_dtypes.float8_e4m3fn

_CACHE = {}


def _build():
    import concourse.bass as bass
    import concourse.tile as tile
    from concourse import bacc, mybir, bass_isa
    from concourse.bass import ts

    f32 = mybir.dt.float32
    bf = mybir.dt.bfloat16
    f8 = mybir.dt.float8e4
    AF = mybir.ActivationFunctionType
    DR = mybir.MatmulPerfMode.DoubleRow
    RADD = bass_isa.ReduceOp.add
    GROUPS = [[0, 1, 2, 3], [4, 5, 6, 7]]

    nc = bacc.Bacc(
        "TRN2",
        target_bir_lowering=False,
        debug=False,
        enable_asserts=True,
        num_devices=NCORES,
    )

    def din(name, shape, dt=bf):
        return nc.dram_tensor(name, shape, dt, kind="ExternalInput").ap()

    # weights pre-tiled on host: contiguous per-tile DMA loads
    x_ap = din("x", [TQ, 128, KC * 512])              # host-pretransposed rows
    qnw_ap = din("qnw8", [NS, 128, KC, 128], f8)      # fp8 norm-shard of q_a
    wq_ap = din("wq", [2, 128, KC, 2 * HEAD_DIM])     # folded q_a@q_b, pair:
    kvaw_ap = din("kvaw", [1 + NR, 128, KC, 128])     # [lat shard | ropes]
    kvbw_ap = din("kvbw", [128, NL, HPC * (QK_NOPE + V_HEAD)])
    ow_ap = din("ow", [128, HPC, HIDDEN])
    mask_ap = din("mask", [128, 128])                 # 0/1 causal block (bf16)
    out_ap = nc.dram_tensor("out", [HIDDEN, T], bf, kind="ExternalOutput").ap()

    def eng(idx):
        return nc.scalar if idx % 2 else nc.vector

    def copy(e, out, in_):
        if e is nc.scalar:
            nc.scalar.copy(out, in_)
        else:
            nc.vector.tensor_copy(out, in_)

    with tile.TileContext(nc) as tc:
        with tc.tile_pool(name="consts", bufs=1) as consts, \
             tc.tile_pool(name="dram", bufs=1, space="DRAM") as dram, \
             tc.tile_pool(name="act", bufs=1) as act, \
             tc.tile_pool(name="owp", bufs=1) as powp:

            mask = consts.tile([128, 128], bf)
            nc.sync.dma_start(out=mask, in_=mask_ap)
            ones128 = consts.tile([128, 128], bf)
            nc.vector.memset(ones128, 1.0)
            eps1 = consts.tile([128, 1], f32)
            nc.vector.memset(eps1, EPS)

            ropes = act.tile([128, NR, T], bf)   # raw rope keys (2 heads/tile)
            rq_b = act.tile([128, T], f32)
            rkv_b = act.tile([128, T], f32)
            rkvT = act.tile([128, TT], f32)
            qn = act.tile([128, HPC, T], bf)
            qrp = act.tile([128, HPC, T], bf)    # per-head rope q, zero-padded
            nc.vector.memset(qrp, 0.0)

            # AllReduce rows: 0 = q-norm partial sumsq, 1 = kv-latent partial
            # sumsq (each core's 128-col shard) — the reduced row 1 IS the
            # full 512-dim latent sumsq, so no post-AllGather recompute.
            # Quarter-major: each quarter rides its own tiny AllReduce,
            # dispatched as soon as that quarter's partials exist, so the
            # rstd chains never wait on the big AllGather.
            ar_in = dram.tile([TQ, 2, 512], f32)
            ar_out = dram.tile([TQ, 2, 512], f32)
            ag_in = dram.tile([128, T], bf)
            ag_out = dram.tile([NL, 128, T], bf)
            rkv_d = dram.tile([1, T], f32)

            # ---- Stages N/KV/QF need xTq alive; freed before attention
            with tc.tile_pool(name="stageA", bufs=1) as pA, \
                 tc.tile_pool(name="wa", bufs=4) as pwa, \
                 tc.tile_pool(name="wD", bufs=1) as pw, \
                 tc.tile_pool(name="pB", bufs=1) as pB, \
                 tc.tile_pool(name="px8", bufs=1) as px8, \
                 tc.tile_pool(name="psumA", bufs=1, space="PSUM") as psumA:
                # x spreads over all 3 DMA queues every quarter; the weight
                # loads are wedged between quarters by need-time. The front
                # is DMA-bandwidth-bound (~170GB/s aggregate over 16 shared
                # engines), so emission order ~= priority.
                def load_xq(quarter):
                    xt = pA.tile([128, KC, 512], bf, name=f"xT{quarter}")
                    if quarter == 0:
                        qs = (nc.sync, nc.scalar, nc.sync, nc.scalar)
                    else:
                        qs = (nc.sync, nc.scalar, nc.gpsimd, nc.gpsimd)
                    for g, q in enumerate(qs):
                        q.dma_start(
                            out=xt[:, 4 * g:4 * g + 4, :],
                            in_=x_ap[quarter, :, ts(g, 4 * 512)],
                        )
                    return xt

                # latent-shard weight + qnw8 lead the gpsimd queue (needed by
                # lat(0)/N(0)); x quarter 0 on sync+scalar; quarters 1-3 put
                # g2/g3 on gpsimd so no single queue carries more than ~1/3
                # of the front traffic
                kva_tiles = [None, None, None]

                def load_kva(n, q):
                    wa = pwa.tile([128, KC, 128], bf, tag="wa", bufs=3)
                    q.dma_start(out=wa, in_=kvaw_ap[n])
                    kva_tiles[n] = wa

                load_kva(0, nc.gpsimd)
                qnw_tiles = []
                for n in range(NS):
                    wa = pwa.tile([128, KC, 128], f8, tag="w8", bufs=3)
                    nc.gpsimd.dma_start(out=wa, in_=qnw_ap[n])
                    qnw_tiles.append(wa)
                xTq = [load_xq(0)]
                load_kva(1, nc.sync)
                load_kva(2, nc.scalar)
                wq_tiles = []
                wq0 = pw.tile([128, KC, 2 * HEAD_DIM], bf, tag="wq",
                              bufs=2, name="wq0")
                nc.sync.dma_start(out=wq0[:, 0:KC // 2, :],
                                  in_=wq_ap[0, :, 0:KC // 2, :])
                nc.scalar.dma_start(out=wq0[:, KC // 2:KC, :],
                                    in_=wq_ap[0, :, KC // 2:KC, :])
                wq_tiles.append(wq0)
                xTq += [load_xq(q) for q in range(1, TQ)]
                wq1 = pw.tile([128, KC, 2 * HEAD_DIM], bf, tag="wq",
                              bufs=2, name="wq1")
                nc.scalar.dma_start(out=wq1, in_=wq_ap[1])
                wq_tiles.append(wq1)
                kvbw = powp.tile([128, NL, HPC * (QK_NOPE + V_HEAD)], bf)
                nc.sync.dma_start(out=kvbw, in_=kvbw_ap)

                # ~40 back-to-back dummy matmuls warm the PE clock (HAM)
                # while the first x/weight DMAs stream in
                warm = psumA.tile([128, 128], f32, tag="warm", bufs=1)
                for w in range(56):
                    nc.tensor.matmul(out=warm, lhsT=ones128, rhs=ones128,
                                     start=(w == 0), stop=(w == 55))

                # ---- Front pipeline, one x-quarter at a time so compute
                # tracks the DMA arrival: latent shard (its sumsq rides the
                # AllReduce as row 1), stage N (fp8 DoubleRow, 2 k-tiles per
                # instruction, x cast to fp8 on Vector/Scalar per 4-ktile
                # group as each DMA lands), rope slices, then QF head-pair 0.
                # The AllGather dispatches right after the last latent write,
                # the AllReduce right after the last stage-N write, so both
                # overlap QF; QF pair 1 runs as a second sweep.
                def qf_quarter(pair, t):
                    wq = wq_tiles[pair]
                    for sub in range(3):  # nope0 | nope1 | rope pair
                        ps = psumA.tile([128, 512], f32, tag="psm", bufs=4)
                        for kk in range(KC):
                            nc.tensor.matmul(
                                out=ps,
                                lhsT=wq[:, kk, ts(sub, 128)],
                                rhs=xTq[t][:, kk, :],
                                start=(kk == 0),
                                stop=(kk == KC - 1),
                            )
                        if sub < 2:
                            nc.scalar.copy(
                                qn[:, 2 * pair + sub, ts(t, 512)], ps)
                        else:
                            # rope halves land zero-padded per head
                            nc.scalar.copy(
                                qrp[0:64, 2 * pair, ts(t, 512)],
                                ps[0:64, :])
                            nc.scalar.copy(
                                qrp[64:128, 2 * pair + 1, ts(t, 512)],
                                ps[64:128, :])

                for t in range(TQ):
                    x8t = px8.tile([128, KC, 512], f8, tag="x8", bufs=2)
                    for g in range(4):
                        ceng = nc.vector if g % 2 == 0 else nc.scalar
                        copy(ceng, x8t[:, 4 * g:4 * g + 4, :],
                             xTq[t][:, 4 * g:4 * g + 4, :])

                    # latent shard quarter (+ its sumsq partial -> ar row 1)
                    psm = psumA.tile([128, 512], f32, tag="psm", bufs=4)
                    for kk in range(KC):
                        nc.tensor.matmul(
                            out=psm,
                            lhsT=kva_tiles[0][:, kk, :],
                            rhs=xTq[t][:, kk, :],
                            start=(kk == 0),
                            stop=(kk == KC - 1),
                        )
                    lst = pB.tile([128, 512], bf, tag="lst", bufs=2)
                    nc.scalar.copy(lst, psm)
                    nc.gpsimd.dma_start(ag_in[:, ts(t, 512)], lst)
                    sqkv = pB.tile([128, 512], bf, tag="sq", bufs=2)
                    nc.scalar.activation(out=sqkv, in_=psm, func=AF.Square)
                    psk = psumA.tile([128, 512], f32, tag="psd", bufs=2)
                    nc.tensor.matmul(out=psk, lhsT=ones128, rhs=sqkv,
                                     start=True, stop=True)
                    sst = pB.tile([1, 512], f32, tag="sst", bufs=2)
                    nc.scalar.copy(sst, psk[0:1, :])
                    nc.gpsimd.dma_start(ar_in[t, 1:2, :], sst)

                    # stage N quarter
                    psd = psumA.tile([128, 512], f32, tag="psd", bufs=2)
                    for n in range(NS):
                        psm = psumA.tile([128, 512], f32, tag="psm", bufs=4)
                        for kk in range(0, KC, 2):
                            nc.tensor.matmul(
                                out=psm,
                                lhsT=qnw_tiles[n][:, kk:kk + 2, :],
                                rhs=x8t[:, kk:kk + 2, :],
                                start=(kk == 0),
                                stop=(kk == KC - 2),
                                perf_mode=DR,
                            )
                        sq = pB.tile([128, 512], bf, tag="sq", bufs=2)
                        nc.scalar.activation(out=sq, in_=psm, func=AF.Square)
                        nc.tensor.matmul(
                            out=psd, lhsT=ones128, rhs=sq,
                            start=(n == 0), stop=(n == NS - 1),
                        )
                    sst = pB.tile([1, 512], f32, tag="sst", bufs=2)
                    nc.scalar.copy(sst, psd[0:1, :])
                    nc.gpsimd.dma_start(ar_in[t, 0:1, :], sst)

                    # this quarter's 4KB AllReduce fires immediately: its
                    # trigger depends only on this quarter's partials, so
                    # every rstd chain has data long before QF ends. The big
                    # AllGather dispatches only after the LAST AllReduce --
                    # a collective queued behind the AllGather on the CC core
                    # eats its rank skew (~20us observed) on top of ~30us of
                    # data movement.
                    nc.gpsimd.collective_compute(
                        "AllReduce",
                        mybir.AluOpType.add,
                        replica_groups=GROUPS,
                        ins=[ar_in[t].opt()],
                        outs=[ar_out[t].opt()],
                    )
                    if t == TQ - 1:
                        nc.gpsimd.collective_compute(
                            "AllGather",
                            mybir.AluOpType.bypass,
                            replica_groups=GROUPS,
                            ins=[ag_in.opt()],
                            outs=[ag_out.opt()],
                        )

                    # rope slices quarter
                    for n in range(1, 1 + NR):
                        psm = psumA.tile([128, 512], f32, tag="psm", bufs=4)
                        for kk in range(KC):
                            nc.tensor.matmul(
                                out=psm,
                                lhsT=kva_tiles[n][:, kk, :],
                                rhs=xTq[t][:, kk, :],
                                start=(kk == 0),
                                stop=(kk == KC - 1),
                            )
                        nc.scalar.copy(ropes[:, n - 1, ts(t, 512)], psm)

                    # QF head-pair 0 for this quarter
                    qf_quarter(0, t)

                # ---- QF head-pair 1 sweep, with the rstd section wedged in
                # after quarter 1 so its DMA->sqrt->recip->broadcast chains
                # (GpSimd/Scalar/Vector, ~25us of serialized latency) fully
                # overlap the remaining QF matmuls on the PE
                qf_quarter(1, 0)
                qf_quarter(1, 1)

                # rstds from the AllReduced sumsq rows: a 1-row f32 ones-
                # matmul broadcasts the raw sums (850ns on the PE -- a GpSimd
                # PartitionBroadcast serializes 2-4us per call), then sqrt +
                # recip on full-height tiles so the chains pipeline across
                # PE/Scalar/Vector ~1us apart. t-major so quarters whose
                # AllReduce finished first flow without waiting on the rest.
                for t in range(TQ):
                    for row, rb in ((0, rq_b), (1, rkv_b)):
                        lora = Q_LORA if row == 0 else KV_LORA
                        rs1 = pB.tile([1, 512], f32, tag="sst", bufs=2)
                        # gpsimd queue: it sits right past the AG trigger, so
                        # these flow as each quarter's AllReduce lands instead
                        # of queueing behind the front x/wq traffic
                        nc.gpsimd.dma_start(rs1[:], ar_out[t, row:row + 1, :])
                        rsb = pB.tile([1, 512], bf, tag="rsb", bufs=2)
                        nc.scalar.copy(rsb, rs1)
                        psb = psumA.tile([128, 512], f32, tag="psd", bufs=2)
                        nc.tensor.matmul(out=psb, lhsT=ones128[0:1, :],
                                         rhs=rsb, start=True, stop=True)
                        t1 = pB.tile([128, 512], f32, tag="t1", bufs=2)
                        nc.scalar.activation(
                            out=t1, in_=psb, func=AF.Sqrt, bias=eps1,
                            scale=1.0 / lora,
                        )
                        nc.vector.reciprocal_approx_fast(
                            out=rb[:, ts(t, 512)], in_=t1)
                        if row == 1:
                            nc.gpsimd.dma_start(rkv_d[0:1, ts(t, 512)],
                                                rb[0:1, ts(t, 512)])
                nc.gpsimd.dma_start(
                    out=rkvT,
                    in_=rkv_d.rearrange("o (tt p) -> (o p) tt", p=128),
                )

                qf_quarter(1, 2)
                qf_quarter(1, 3)

                for t in range(TQ):
                    for h in range(HPC):
                        nc.vector.tensor_mul(
                            qn[:, h, ts(t, 512)], qn[:, h, ts(t, 512)],
                            rq_b[:, ts(t, 512)],
                        )
                    for h in range(HPC):
                        hh = 64 * (h % 2)
                        nc.vector.tensor_mul(
                            qrp[hh:hh + 64, h, ts(t, 512)],
                            qrp[hh:hh + 64, h, ts(t, 512)],
                            rq_b[hh:hh + 64, ts(t, 512)],
                        )

            # ---- Stages D/E/F
            with tc.tile_pool(name="att", bufs=1) as patt, \
                 tc.tile_pool(name="trans", bufs=3) as trans, \
                 tc.tile_pool(name="owd", bufs=1) as powd, \
                 tc.tile_pool(name="psumD", bufs=1, space="PSUM") as psumD:
                kn = patt.tile([128, HPC, T], bf)
                vv = patt.tile([128, TT, HPC * V_HEAD], bf)
                xlat = patt.tile([128, NL, T], bf)   # AllGathered raw latent
                for kk in range(NL):
                    # halves on sync+scalar; gpsimd stays free for the
                    # rstd broadcasts and then the o_w load
                    nc.sync.dma_start(out=xlat[:, kk, 0:1024],
                                      in_=ag_out[kk][:, 0:1024])
                    nc.scalar.dma_start(out=xlat[:, kk, 1024:2048],
                                        in_=ag_out[kk][:, 1024:2048])
                ow = powd.tile([128, HPC, HIDDEN], bf)
                nc.gpsimd.dma_start(out=ow, in_=ow_ap)

                # ---- Stage D: k_nope (x rstd_kv), v (x rstd_kv)
                for h in range(HPC):
                    for t in range(TQ):
                        ps3 = psumD.tile([128, 512], f32, tag="psm", bufs=2)
                        for kk in range(NL):
                            nc.tensor.matmul(
                                out=ps3,
                                lhsT=kvbw[:, kk, ts(h, 256)][:, 0:128],
                                rhs=xlat[:, kk, ts(t, 512)],
                                start=(kk == 0),
                                stop=(kk == NL - 1),
                            )
                        nc.vector.tensor_mul(
                            kn[:, h, ts(t, 512)], ps3, rkv_b[:, ts(t, 512)]
                        )
                vcols = kvbw.rearrange(
                    "p kk (h two dv) -> p kk h two dv", h=HPC, two=2
                )
                for tt in range(TT):
                    psv = psumD.tile([128, 512], f32, tag="psm", bufs=2)
                    for kk in range(NL):
                        nc.tensor.matmul(
                            out=psv,
                            lhsT=xlat[:, kk, ts(tt, 128)],
                            rhs=vcols[:, kk, :, 1, :],
                            start=(kk == 0),
                            stop=(kk == NL - 1),
                        )
                    nc.scalar.mul(vv[:, tt, :], psv, mul=rkvT[:, tt:tt + 1])

                # ---- Stage E+F: causal attention; o_proj one chunk behind
                with tc.tile_pool(name="attn_i", bufs=2) as pai, \
                     tc.tile_pool(name="ob", bufs=2) as pob, \
                     tc.tile_pool(name="rdb", bufs=2) as prdb:
                    attn_tiles = {}

                    def attention_chunk(i):
                        attn_i = pai.tile([128, HPC, 512], bf, tag="attn_i", bufs=2)
                        for h in range(HPC):
                            nj = 4 * i + 4
                            pso = psumD.tile([128, 512], f32, tag="pso", bufs=3)
                            # exp-sum accumulated on VectorE in bf16 (2x DVE
                            # rate; the partition-reduce upcasts to f32);
                            # GpSimd turns it into the softmax denominator
                            acc = prdb.tile([128, 512], bf, tag="acc", bufs=2)

                            def consume_batch(batch, last):
                                for jc, exc, qb in batch:
                                    nc.tensor.matmul(
                                        out=pso[:, qb:512],
                                        lhsT=vv[:, jc, ts(h, V_HEAD)],
                                        rhs=exc[:, qb:512],
                                        start=(jc == 0),
                                        stop=(last and jc == batch[-1][0]),
                                    )

                            pending = []
                            for j in range(nj):
                                off = j * 128 - i * 512
                                qb = max(0, off)  # 1st query col this key sees
                                pss = psumD.tile([128, 512], f32, tag="pss", bufs=3)
                                nc.tensor.matmul(
                                    out=pss[:, qb:512],
                                    lhsT=kn[:, h, ts(j, 128)],
                                    rhs=qn[:, h, i * 512 + qb:(i + 1) * 512],
                                    start=True,
                                    stop=False,
                                )
                                nc.tensor.matmul(
                                    out=pss[:, qb:512],
                                    lhsT=ropes[:, h // 2, ts(j, 128)],
                                    rhs=qrp[:, h, i * 512 + qb:(i + 1) * 512],
                                    start=False,
                                    stop=True,
                                )
                                if len(pending) == 4:
                                    consume_batch(pending, False)
                                    pending = []
                                ex = trans.tile([128, 512], bf, tag="ex", bufs=6)
                                nc.scalar.activation(
                                    out=ex[:, qb:512], in_=pss[:, qb:512],
                                    func=AF.Exp,
                                )
                                if off >= 0:
                                    nc.vector.tensor_mul(
                                        ex[:, qb:qb + 128], ex[:, qb:qb + 128],
                                        mask,
                                    )
                                if j == 0:
                                    nc.vector.tensor_copy(acc, ex)
                                else:
                                    nc.vector.tensor_add(
                                        acc[:, qb:512], acc[:, qb:512],
                                        ex[:, qb:512],
                                    )
                                pending.append((j, ex, qb))
                            if pending:
                                consume_batch(pending, True)

                            # denominator: one cheap bf16 ones-matmul (263ns,
                            # ~1.8us total latency after the last j) -- a
                            # GpSimd PartitionAllReduce here costs 3.8us of
                            # latency and stalls the pso ring every head
                            psdn = psumD.tile([128, 512], f32, tag="psm",
                                              bufs=2)
                            nc.tensor.matmul(out=psdn, lhsT=ones128, rhs=acc,
                                             start=True, stop=True)
                            rdb = prdb.tile([128, 512], f32, tag="rdb", bufs=2)
                            nc.vector.reciprocal_approx_fast(out=rdb, in_=psdn)
                            nc.vector.tensor_mul(attn_i[:, h, :], pso, rdb)
                        attn_tiles[i] = attn_i

                    def oproj_chunk(i):
                        attn_i = attn_tiles[i]
                        for m in range(TT):
                            psf = psumD.tile([128, 512], f32, tag="psm", bufs=2)
                            for kk in range(HPC):
                                nc.tensor.matmul(
                                    out=psf,
                                    lhsT=ow[:, kk, ts(m, 128)],
                                    rhs=attn_i[:, kk, :],
                                    start=(kk == 0),
                                    stop=(kk == HPC - 1),
                                )
                            ob = pob.tile([128, 512], bf, tag="ob", bufs=3)
                            # alternate Scalar/Vector: exp leaves ~half the
                            # ACT engine free during the o_proj windows
                            copy(eng(m), ob, psf)
                            (nc.sync if m % 2 else nc.scalar).dma_start(
                                out=out_ap[ts(m, 128), ts(i, 512)], in_=ob
                            )

                    # big chunks first: the tail then drains on the
                    # smallest attention chunk + its o_proj
                    order = list(range(TQ))[::-1]
                    attention_chunk(order[0])
                    for k in range(1, TQ):
                        attention_chunk(order[k])
                        oproj_chunk(order[k - 1])
                    oproj_chunk(order[-1])

    nc.compile()
    return nc


def _tile_w(w, dt=BF16):
    """[K, N] -> [N/128, 128, K/128, 128] so each col-block loads contiguously."""
    K, N = w.shape
    return np.ascontiguousarray(
        w.astype(dt).reshape(K // 128, 128, N // 128, 128).transpose(2, 1, 0, 3))


def _prep(inputs):
    x = np.asarray(inputs["hidden_states"], np.float32)
    qaw = np.asarray(inputs["q_a_w"], np.float32)
    qalw = np.asarray(inputs["q_a_ln_w"], np.float32)
    qbw = np.asarray(inputs["q_b_w"], np.float32)
    kvaw = np.asarray(inputs["kv_a_w"], np.float32)
    kvlw = np.asarray(inputs["kv_a_ln_w"], np.float32)
    kvbw = np.asarray(inputs["kv_b_w"], np.float32)
    ow = np.asarray(inputs["o_w"], np.float32)

    scale = 1.0 / np.sqrt(np.float32(HEAD_DIM))
    qbw_f = qbw * qalw[:, None] * scale
    wq_full = qaw @ qbw_f  # [HIDDEN, NUM_HEADS*HEAD_DIM] fp32 fold
    kvbw_f = (kvbw * kvlw[:, None]).astype(BF16)

    r = np.arange(128)[:, None]
    j = np.arange(128)[None, :]
    mask = np.where(j >= r, 1.0, 0.0).astype(BF16)

    def lat_tiled(w):  # [KV_LORA, N] -> [128, NL, N] (p, kk, n)
        return np.ascontiguousarray(w.reshape(NL, 128, -1).transpose(1, 0, 2))

    in_maps = []
    for c in range(NCORES):
        b, g = c // 4, c % 4
        # folded W_q for this head group, pair layout [nope0|nope1|ropes]
        wq_g = wq_full[:, g * HPC * HEAD_DIM:(g + 1) * HPC * HEAD_DIM]
        pairs = []
        for pair in range(HPC // 2):
            h0, h1 = 2 * pair, 2 * pair + 1
            cols = np.concatenate([
                wq_g[:, h0 * HEAD_DIM:h0 * HEAD_DIM + QK_NOPE],
                wq_g[:, h1 * HEAD_DIM:h1 * HEAD_DIM + QK_NOPE],
                wq_g[:, h0 * HEAD_DIM + QK_NOPE:(h0 + 1) * HEAD_DIM],
                wq_g[:, h1 * HEAD_DIM + QK_NOPE:(h1 + 1) * HEAD_DIM],
            ], axis=1).astype(BF16)  # [HIDDEN, 384]
            pairs.append(cols.reshape(KC, 128, 384).transpose(1, 0, 2))
        wq_c = np.ascontiguousarray(np.stack(pairs))   # [2, 128, KC, 384]

        # fp8 norm-shard of q_a: 384 cols per core
        qnw8 = _tile_w(np.ascontiguousarray(
            qaw[:, g * 384:(g + 1) * 384]), FP8)

        # kv_a: this core's 128-col latent shard + its 4 heads' rope cols
        kvaw_g = np.concatenate(
            [kvaw[:, g * 128:(g + 1) * 128],
             kvaw[:, KV_LORA + g * HPC * QK_ROPE:
                  KV_LORA + (g + 1) * HPC * QK_ROPE]], axis=1).astype(BF16)

        xt = np.ascontiguousarray(
            x[b].T.astype(BF16).reshape(KC, 128, TQ, 512).transpose(2, 1, 0, 3)
            .reshape(TQ, 128, KC * 512))
        in_maps.append({
            "x": xt,
            "qnw8": qnw8,
            "wq": wq_c,
            "kvaw": _tile_w(kvaw_g),
            "kvbw": lat_tiled(kvbw_f[:, g * HPC * 256:(g + 1) * HPC * 256]),
            "ow": np.ascontiguousarray(
                ow[g * HPC * V_HEAD:(g + 1) * HPC * V_HEAD]
                .astype(BF16).reshape(HPC, 128, HIDDEN).transpose(1, 0, 2)),
            "mask": mask,
        })
    return in_maps


def _ensure_trace_shim():
    """This image lacks antenv.axon_hooks; synthesize it so a trace=True (or
    BASS_TRACE=1) invocation degrades gracefully instead of crashing."""
    import sys
    import types
    try:
        import antenv.axon_hooks  # noqa: F401
        return
    except Exception:
        pass
    try:
        import antenv
        import trn_agent_boot.trn_boot as tb
        hook = tb._ntff_profile_via_ctypes("/opt/axon/libaxon_pjrt.so")
        mod = types.ModuleType("antenv.axon_hooks")
        mod.get_axon_ntff_profile_hook = lambda: hook
        mod.set_axon_ntff_profile_hook = lambda h: None
        antenv.axon_hooks = mod
        sys.modules["antenv.axon_hooks"] = mod
        import concourse.bass_utils as bu
        bu.upload_artifacts = lambda tmpdir: tmpdir
    except Exception:
        pass


def kernel(**inputs):
    from concourse.bass_utils import run_bass_kernel_spmd

    _ensure_trace_shim()
    if "nc" not in _CACHE:
        _CACHE["nc"] = _build()
    nc = _CACHE["nc"]
    in_maps = _prep(inputs)
    try:
        res = run_bass_kernel_spmd(nc, in_maps, core_ids=list(range(NCORES)),
                                   **_CACHE.get("run_kwargs", {}))
    except Exception:
        # transient accelerator faults (e.g. NRT_EXEC_UNIT_UNRECOVERABLE) have
        # been observed after interrupted runs; one retry clears them
        import time
        time.sleep(2)
        res = run_bass_kernel_spmd(nc, in_maps, core_ids=list(range(NCORES)),
                                   **_CACHE.get("run_kwargs", {}))
    _CACHE["last_results"] = res
    out = np.zeros((B, T, HIDDEN), np.float32)
    for c in range(NCORES):
        out[c // 4] += np.asarray(res.results[c]["out"], np.float32).T
    return out
